# revision 12
# baseline (speedup 1.0000x reference)
"""Trainium2 Bass kernel for nn_Net3DSeg (gnn_message_passing).

Data-parallel over B=4 samples x 2 row-halves = 8 cores. Each core handles
4096 query points of one sample (against all 8192 candidate columns):
  - seg_logit / feats_ssp / seg_logit2 rows (matmuls on PE, biases folded in)
  - 12-NN graph via packed-key chunked top-k:
      key[p,j] = (bits(-d[p,j]) & 0xFFFFE000) | quantized_gray[col j]
    max8 per 1024-col chunk -> 64 candidates -> max8/match_replace/max8
    gives the top-12 keys in ascending-distance order; the low 13 bits carry
    the neighbor's quantized gray, so no gather is needed.
  - partial abs-sum loss terms, reduced on host (scalar-only allreduce).

The candidate column order is rotated per core (host-side) so that the
core's own 4096 rows are always columns 0..4095 -> a single SPMD program.
"""

import numpy as np

import concourse.bass as bass
import concourse.bacc as bacc
import concourse.mybir as mybir
from concourse.tile import TileContext
from concourse.masks import make_identity
from concourse.bass_utils import run_bass_kernel_spmd

B, P, C, NC, K = 4, 8192, 64, 10, 12
H = W = 256
HALF = P // 2          # rows per core: 4096
NT = HALF // 128       # row tiles per core: 32
NGT = P // 128         # gather tiles (full sample): 64
CHUNK = 1024           # KNN column chunk
NCH = P // CHUNK       # chunks: 8
MASK_HI = 0xFFFFE000
MASK_LO = 0x00001FFF
G_LO, G_HI = -3.5, 3.5
G_SCALE = 8191.0 / (G_HI - G_LO)       # gray -> payload
G_INV = (G_HI - G_LO) / 8191.0         # payload -> gray
NEG_BIG = -3.4028234663852886e38
F32 = mybir.dt.float32
U32 = mybir.dt.uint32

_CACHE = {}


def build_program():
    nc = bacc.Bacc()
    dp = nc.declare_dram_parameter
    feats_in = dp("feats", [HALF, C], F32, isOutput=False)
    xyzT_in = dp("xyzT", [3, P], F32, isOutput=False)
    imgT_in = dp("imgT", [H * W, 3], F32, isOutput=False)
    ii_in = dp("ii", [P, 2], U32, isOutput=False)
    w3a_in = dp("w3a", [C + 1, C], F32, isOutput=False)
    w4_in = dp("w4", [C, C], F32, isOutput=False)
    b4_in = dp("b4", [C, 1], F32, isOutput=False)
    w5_in = dp("w5", [C, 3 + K], F32, isOutput=False)
    wl1_in = dp("wl1", [C + 1, NC], F32, isOutput=False)
    wl2_in = dp("wl2", [C + 1, NC], F32, isOutput=False)
    wgray_in = dp("wgray", [1, 3 * NGT], F32, isOutput=False)
    crow_in = dp("crow", [2, P], F32, isOutput=False)

    seg1_out = dp("seg1", [HALF, NC], F32, isOutput=True)
    fssp_out = dp("fssp", [HALF, C], F32, isOutput=True)
    seg2_out = dp("seg2", [HALF, NC], F32, isOutput=True)
    lpart_out = dp("lpart", [128, 2], F32, isOutput=True)

    qg_dram = nc.dram_tensor("qg_stage", [1, P], U32)

    AND = mybir.AluOpType.bitwise_and
    OR = mybir.AluOpType.bitwise_or
    ADD = mybir.AluOpType.add
    SUB = mybir.AluOpType.subtract
    MUL = mybir.AluOpType.mult
    ABSMAX = mybir.AluOpType.abs_max

    with TileContext(nc) as tc:
        with (
            tc.tile_pool(name="persist", bufs=1) as pers,
            tc.tile_pool(name="sb", bufs=2) as sb,
            tc.tile_pool(name="keys", bufs=3) as kp,
            tc.tile_pool(name="small", bufs=3) as sm,
            tc.tile_pool(name="pdist", bufs=2, space="PSUM") as pdist,
            tc.tile_pool(name="plin", bufs=3, space="PSUM") as plin,
        ):
            # ---------------- one-time prep ----------------
            ident = pers.tile([128, 128], F32, tag="ident")
            make_identity(nc, ident[:])

            w3a = pers.tile([C + 1, C], F32, tag="w3a")
            w4 = pers.tile([C, C], F32, tag="w4")
            b4 = pers.tile([C, 1], F32, tag="b4")
            w5 = pers.tile([C, 3 + K], F32, tag="w5")
            wl1 = pers.tile([C + 1, NC], F32, tag="wl1")
            wl2 = pers.tile([C + 1, NC], F32, tag="wl2")
            for t_, src in ((w3a, w3a_in), (w4, w4_in), (b4, b4_in),
                            (w5, w5_in), (wl1, wl1_in), (wl2, wl2_in)):
                nc.sync.dma_start(out=t_[:], in_=src[:])

            # xyzT -> lhsT staging [5, P] (rows: 2x, sq, 1) and
            # rhs staging [5, P] (rows: x, -1, -sq); -d = lhsT.T @ rhs
            biglhsT = pers.tile([5, P], F32, tag="biglhsT")
            bigrhs = pers.tile([5, P], F32, tag="bigrhs")
            nc.sync.dma_start(out=bigrhs[0:3, :], in_=xyzT_in[:])
            nc.vector.tensor_scalar(
                out=biglhsT[0:3, :], in0=bigrhs[0:3, :], scalar1=2.0,
                scalar2=None, op0=MUL)
            # constant rows -1 / +1 from host
            nc.sync.dma_start(out=bigrhs[3:4, :], in_=crow_in[0:1, :])
            nc.sync.dma_start(out=biglhsT[4:5, :], in_=crow_in[1:2, :])
            # sq rows, computed in 512-col chunks
            negones3 = pers.tile([3, 1], F32, tag="negones3")
            nc.vector.memset(negones3[:], -1.0)
            for i in range(P // 512):
                sl = slice(i * 512, (i + 1) * 512)
                xs = sm.tile([3, 512], F32, tag="xs")
                nc.vector.tensor_tensor(
                    out=xs[:], in0=bigrhs[0:3, sl], in1=bigrhs[0:3, sl],
                    op=MUL)
                ps = plin.tile([1, 512], F32, space="PSUM", tag="lin")
                nc.tensor.matmul(ps[:], lhsT=negones3[:], rhs=xs[:],
                                 start=True, stop=True)
                sqn = sm.tile([1, 512], F32, tag="sqn")
                nc.scalar.copy(out=sqn[:], in_=ps[:])
                sqp = sm.tile([1, 512], F32, tag="sqp")
                nc.vector.tensor_scalar(
                    out=sqp[:], in0=sqn[:], scalar1=-1.0, scalar2=None,
                    op0=MUL)
                nc.sync.dma_start(out=bigrhs[4:5, sl], in_=sqn[:])
                nc.sync.dma_start(out=biglhsT[3:4, sl], in_=sqp[:])

            # image gather: lin = u*256 + v, gather rgb rows, gray, qg
            iit = pers.tile([128, NGT * 2], U32, tag="iit")
            nc.sync.dma_start(
                out=iit[:].rearrange("p (t c) -> p t c", c=2),
                in_=ii_in[:].rearrange("(t p) c -> p t c", p=128))
            iit3 = iit[:].rearrange("p (t c) -> p t c", c=2)
            lin = pers.tile([128, NGT], U32, tag="lin_idx")
            nc.vector.tensor_scalar(
                out=lin[:], in0=iit3[:, :, 0], scalar1=256.0, scalar2=None,
                op0=MUL)
            nc.vector.tensor_tensor(
                out=lin[:], in0=lin[:], in1=iit3[:, :, 1], op=ADD)

            rgball = pers.tile([128, NGT * 3], F32, tag="rgball")
            for t in range(NGT):
                nc.gpsimd.indirect_dma_start(
                    out=rgball[:, t * 3:(t + 1) * 3], out_offset=None,
                    in_=imgT_in[:],
                    in_offset=bass.IndirectOffsetOnAxis(
                        ap=lin[:, t:t + 1], axis=0))

            wgrayrep = pers.tile([128, 3], F32, tag="wgrayrep")
            nc.sync.dma_start(out=wgrayrep[:],
                              in_=wgray_in[:, 0:3].to_broadcast([128, 3]))
            gray = pers.tile([128, NGT], F32, tag="gray")
            # per-gather-tile to keep per-instruction sync waits low
            for t in range(NGT):
                gp = sm.tile([128, 3], F32, tag="gp")
                nc.vector.tensor_tensor(
                    out=gp[:], in0=rgball[:, t * 3:(t + 1) * 3],
                    in1=wgrayrep[:], op=MUL)
                nc.vector.tensor_reduce(
                    out=gray[:, t:t + 1], in_=gp[:],
                    axis=mybir.AxisListType.X, op=ADD)

            qgf = pers.tile([128, NGT], F32, tag="qgf")
            nc.vector.tensor_scalar(
                out=qgf[:], in0=gray[:], scalar1=G_SCALE,
                scalar2=-G_LO * G_SCALE + 0.5, op0=MUL, op1=ADD)
            nc.vector.tensor_scalar(
                out=qgf[:], in0=qgf[:], scalar1=0.0, scalar2=8191.0,
                op0=mybir.AluOpType.max, op1=mybir.AluOpType.min)
            qgu = pers.tile([128, NGT], U32, tag="qgu")
            nc.vector.tensor_copy(out=qgu[:], in_=qgf[:])
            nc.sync.dma_start(
                out=qg_dram[:].rearrange("o (t p) -> (o p) t", p=128),
                in_=qgu[:])
            qgrep = pers.tile([128, P], U32, tag="qgrep")
            nc.sync.dma_start(out=qgrep[:],
                              in_=qg_dram[:].to_broadcast([128, P]))

            # persistent augmented-lhsT staging (ones row at index C)
            ftaug = [pers.tile([C + 1, 128], F32, tag=f"ftaug{i}",
                                name=f"ftaug{i}") for i in range(2)]
            h1aug = [pers.tile([C + 1, 128], F32, tag=f"h1aug{i}",
                                name=f"h1aug{i}") for i in range(2)]
            for tl in ftaug + h1aug:
                nc.vector.memset(tl[C:C + 1, :], 1.0)

            accG = pers.tile([128, K], F32, tag="accG")
            accR = pers.tile([128, 3], F32, tag="accR")
            nc.vector.memset(accG[:], 0.0)
            nc.vector.memset(accR[:], 0.0)

            # ---------------- main loop over row tiles ----------------
            for t in range(NT):
                rows = slice(t * 128, (t + 1) * 128)
                fta = ftaug[t % 2]
                h1a = h1aug[t % 2]

                # ---- linear heads ----
                ft = sb.tile([128, C], F32, tag="ft")
                nc.sync.dma_start(out=ft[:], in_=feats_in[rows, :])
                ftT_ps = plin.tile([C, 128], F32, space="PSUM", tag="lin")
                nc.tensor.transpose(out=ftT_ps[:], in_=ft[:],
                                    identity=ident[:])
                nc.scalar.copy(out=fta[0:C, :], in_=ftT_ps[:])

                h1_ps = plin.tile([128, C], F32, space="PSUM", tag="lin")
                nc.tensor.matmul(h1_ps[:], lhsT=fta[:], rhs=w3a[:],
                                 start=True, stop=True)
                h1 = sb.tile([128, C], F32, tag="h1")
                nc.scalar.copy(out=h1[:], in_=h1_ps[:])
                nc.sync.dma_start(out=fssp_out[rows, :], in_=h1[:])

                h1T_ps = plin.tile([C, 128], F32, space="PSUM", tag="lin")
                nc.tensor.transpose(out=h1T_ps[:], in_=h1[:],
                                    identity=ident[:])
                nc.scalar.copy(out=h1a[0:C, :], in_=h1T_ps[:])

                h2T_ps = plin.tile([C, 128], F32, space="PSUM", tag="lin")
                nc.tensor.matmul(h2T_ps[:], lhsT=w4[:], rhs=h1a[0:C, :],
                                 start=True, stop=True)
                h2T = sb.tile([C, 128], F32, tag="h2T")
                nc.vector.tensor_scalar(
                    out=h2T[:], in0=h2T_ps[:], scalar1=b4[:, :1],
                    scalar2=None, op0=ADD)

                pre3_ps = plin.tile([128, 3 + K], F32, space="PSUM",
                                    tag="lin")
                nc.tensor.matmul(pre3_ps[:], lhsT=h2T[:], rhs=w5[:],
                                 start=True, stop=True)
                pre3 = sb.tile([128, 3 + K], F32, tag="pre3")
                nc.scalar.copy(out=pre3[:], in_=pre3_ps[:])

                s1_ps = plin.tile([128, NC], F32, space="PSUM", tag="lin")
                nc.tensor.matmul(s1_ps[:], lhsT=fta[:], rhs=wl1[:],
                                 start=True, stop=True)
                s1 = sb.tile([128, NC], F32, tag="s1")
                nc.scalar.copy(out=s1[:], in_=s1_ps[:])
                nc.sync.dma_start(out=seg1_out[rows, :], in_=s1[:])

                s2_ps = plin.tile([128, NC], F32, space="PSUM", tag="lin")
                nc.tensor.matmul(s2_ps[:], lhsT=h1a[:], rhs=wl2[:],
                                 start=True, stop=True)
                s2 = sb.tile([128, NC], F32, tag="s2")
                nc.scalar.copy(out=s2[:], in_=s2_ps[:])
                nc.sync.dma_start(out=seg2_out[rows, :], in_=s2[:])

                # ---- KNN: packed-key chunked top-k ----
                lhs_t = biglhsT[:, t * 128:(t + 1) * 128]
                cand = sm.tile([128, NCH * 8], F32, tag="cand")
                for cg in range(NCH):
                    cs = cg * CHUNK
                    dps = pdist.tile([128, CHUNK], F32, space="PSUM",
                                     tag="dist")
                    nc.tensor.matmul(dps[:, 0:512], lhsT=lhs_t,
                                     rhs=bigrhs[:, cs:cs + 512],
                                     start=True, stop=True)
                    nc.tensor.matmul(dps[:, 512:1024], lhsT=lhs_t,
                                     rhs=bigrhs[:, cs + 512:cs + 1024],
                                     start=True, stop=True)
                    keys = kp.tile([128, CHUNK], U32, tag="keys")
                    nc.vector.tensor_scalar(
                        out=keys[:], in0=dps[:].bitcast(U32),
                        scalar1=MASK_HI, scalar2=None, op0=AND)
                    nc.vector.tensor_tensor(
                        out=keys[:], in0=keys[:],
                        in1=qgrep[:, cs:cs + CHUNK], op=OR)
                    nc.vector.max(out=cand[:, cg * 8:(cg + 1) * 8],
                                  in_=keys[:].bitcast(F32))

                top8a = sm.tile([128, 8], F32, tag="top8a")
                top8b = sm.tile([128, 8], F32, tag="top8b")
                candr = sm.tile([128, NCH * 8], F32, tag="candr")
                nc.vector.max(out=top8a[:], in_=cand[:])
                nc.vector.match_replace(out=candr[:], in_to_replace=top8a[:],
                                        in_values=cand[:],
                                        imm_value=NEG_BIG)
                nc.vector.max(out=top8b[:], in_=candr[:])

                qq = sm.tile([128, K], U32, tag="qq")
                nc.vector.tensor_scalar(
                    out=qq[:, 0:8], in0=top8a[:].bitcast(U32),
                    scalar1=MASK_LO, scalar2=None, op0=AND)
                nc.vector.tensor_scalar(
                    out=qq[:, 8:K], in0=top8b[:, 0:4].bitcast(U32),
                    scalar1=MASK_LO, scalar2=None, op0=AND)
                qf = sm.tile([128, K], F32, tag="qf")
                nc.vector.tensor_copy(out=qf[:], in_=qq[:])

                # |(qf*G_INV + (G_LO - gself)) - local12| summed into accG
                gsneg = sm.tile([128, 1], F32, tag="gsneg")
                nc.vector.tensor_scalar(
                    out=gsneg[:], in0=gray[:, t:t + 1], scalar1=-1.0,
                    scalar2=G_LO, op0=MUL, op1=ADD)
                d12 = sm.tile([128, K], F32, tag="d12")
                nc.vector.tensor_scalar(
                    out=d12[:], in0=qf[:], scalar1=G_INV,
                    scalar2=gsneg[:, :1], op0=MUL, op1=ADD)
                nc.vector.tensor_tensor(
                    out=d12[:], in0=d12[:], in1=pre3[:, 3:3 + K], op=SUB)
                nc.vector.tensor_scalar(
                    out=d12[:].bitcast(U32), in0=d12[:].bitcast(U32),
                    scalar1=0x7FFFFFFF, scalar2=None, op0=AND)
                nc.vector.tensor_tensor(
                    out=accG[:], in0=accG[:], in1=d12[:], op=ADD)

                # |rgb_pre - rgb| summed into accR
                d3 = sm.tile([128, 3], F32, tag="d3")
                nc.vector.tensor_tensor(
                    out=d3[:], in0=pre3[:, 0:3],
                    in1=rgball[:, t * 3:(t + 1) * 3], op=SUB)
                nc.vector.tensor_scalar(
                    out=d3[:].bitcast(U32), in0=d3[:].bitcast(U32),
                    scalar1=0x7FFFFFFF, scalar2=None, op0=AND)
                nc.vector.tensor_tensor(
                    out=accR[:], in0=accR[:], in1=d3[:], op=ADD)

            # ---------------- loss partials out ----------------
            lpart = pers.tile([128, 2], F32, tag="lpart")
            nc.vector.tensor_reduce(
                out=lpart[:, 0:1], in_=accR[:], axis=mybir.AxisListType.X,
                op=ADD)
            nc.vector.tensor_reduce(
                out=lpart[:, 1:2], in_=accG[:], axis=mybir.AxisListType.X,
                op=ADD)
            nc.sync.dma_start(out=lpart_out[:], in_=lpart[:])

    nc.compile()
    return nc


def get_program():
    if "prog" not in _CACHE:
        _CACHE["prog"] = build_program()
    return _CACHE["prog"]


def prepare_in_maps(feats, xyz, img, img_indices, W_lin, b_lin, W_lin2,
                    b_lin2, W3, b3, W4, b4, W5):
    f32 = lambda a: np.ascontiguousarray(np.asarray(a, dtype=np.float32))
    feats = f32(feats)
    xyz = f32(xyz)
    img = f32(img)
    ii = np.asarray(img_indices).astype(np.uint32)
    w3a = np.concatenate([f32(W3), f32(b3)[None, :]], axis=0)
    wl1 = np.concatenate([f32(W_lin), f32(b_lin)[None, :]], axis=0)
    wl2 = np.concatenate([f32(W_lin2), f32(b_lin2)[None, :]], axis=0)
    wgray = np.ascontiguousarray(
        np.tile(np.array([0.299, 0.587, 0.114], np.float32), NGT)[None, :])
    crow = np.ascontiguousarray(np.stack([
        np.full((P,), -1.0, np.float32), np.full((P,), 1.0, np.float32)]))
    b4c = np.ascontiguousarray(f32(b4)[:, None])
    w4_ = f32(W4)
    w5_ = f32(W5)

    in_maps = []
    for c in range(8):
        b, h = c // 2, c % 2
        # rotate candidate columns so this core's own rows are cols 0..4095
        rot = np.roll(np.arange(P), -h * HALF)
        in_maps.append({
            "feats": np.ascontiguousarray(
                feats[b * P + h * HALF: b * P + (h + 1) * HALF]),
            "xyzT": np.ascontiguousarray(xyz[b][rot].T),
            "imgT": np.ascontiguousarray(img[b].reshape(3, H * W).T),
            "ii": np.ascontiguousarray(ii[b][rot]),
            "w3a": w3a, "w4": w4_, "b4": b4c, "w5": w5_,
            "wl1": wl1, "wl2": wl2, "wgray": wgray, "crow": crow,
        })
    return in_maps


def combine_outputs(results):
    seg1 = np.concatenate([np.asarray(r["seg1"]) for r in results], axis=0)
    fssp = np.concatenate([np.asarray(r["fssp"]) for r in results], axis=0)
    seg2 = np.concatenate([np.asarray(r["seg2"]) for r in results], axis=0)
    s_rgb = np.float64(0.0)
    s_grad = np.float64(0.0)
    for r in results:
        lp = np.asarray(r["lpart"], dtype=np.float64)
        s_rgb += lp[:, 0].sum()
        s_grad += lp[:, 1].sum()
    N = B * P
    loss = np.float32(s_rgb / (N * 3) + 0.1 * (s_grad / (N * K)))
    return seg1, fssp, loss, seg2


def kernel(**inputs):
    in_maps = prepare_in_maps(**inputs)
    nc = get_program()
    res = run_bass_kernel_spmd(nc, in_maps, core_ids=list(range(8)))
    return combine_outputs(res.results)


def kernel_timed(inputs, trace=True):
    """Like kernel() but returns (outputs, exec_time_ns, profile_results)."""
    in_maps = prepare_in_maps(**inputs)
    nc = get_program()
    res = run_bass_kernel_spmd(nc, in_maps, core_ids=list(range(8)),
                               trace=trace)
    return combine_outputs(res.results), res.exec_time_ns, res
